# revision 34
# baseline (speedup 1.0000x reference)
"""Distributed GQA attention kernel for 8 TRN2 NeuronCores.

Problem: B=2, S=2048, D=2048, H=32 heads, KVH=4 kv-heads, HD=64 (GQA),
RoPE + causal attention + output projection, fp32 inputs/outputs.

Sharding: tensor-parallel over heads. Core c owns q-heads [4c..4c+4) and
kv-head c//2 (each kv head is shared by 2 cores; its tiny K/V projection is
recomputed on both). Per core:
  1. QKV projection from the replicated, host-pre-transposed x^T (bf16) with
     the core's weight column slice packed as one [2048, 384] bf16 rhs
     (256 q | 64 k | 64 v).
  2. RoPE in natural layout on the DVE (weight columns de-interleaved on host
     so each head is [32 reals | 32 imags]; q.k is invariant under a common
     permutation of head dims).
  3. Q,K transposed on the PE; K's [64,128] transpose lands in TWO
     zero-padded kt variants (K^T on one partition half, zeros on the other)
     so scores contract over all 128 partitions of the two-head q tile and
     every PE instruction stays in 128x128 tile mode (no mode-switch drains).
     Scores are computed transposed (scoresT[kpos, q]) so the softmax
     normalizer falls out of ones-columns appended to V in the PV matmul.
  4. Causal flash attention in bf16, head pairs interleaved. Per q chunk:
     two packed diagonal groups (fully-masked leading columns of each kpos
     chunk dropped from scores/exp/PV; one strided 0/1 triangle multiply per
     group) run first, then clean kpos-chunk pairs descending, software-
     pipelined one group deep so exps overlap the next group's scores.
  5. Normalization entirely on DVE: 64 replicated ones-columns in the PV
     weights land the sums in psum rows 64:128; copy to base-0 SBUF (custom-
     DVE recip requires aligned bases), 64-lane reciprocal, one multiply
     straight from psum. (gpsimd carries ONLY collectives + at-tile loads,
     so batch-1 compute never queues behind the batch-0 AllToAll.)
  6. Attention outputs staged (transposed) to DRAM in AllToAll layout; TWO
     half-collectives per batch (heads 0-1, then heads 2-3) so comm starts
     halfway through each batch's attention and the final collective only
     carries 0.5 MB.
  7. Row-sharded output projection (rows [256c..256c+256) of each batch)
     against the fully-resident bf16 wo. Batch-0 rows are pinned only after
     batch-1's first head-pair so they fill the ACT-bound second half;
     batch-1 rows run as even-k partial chains (drained to SBUF bf16) DURING
     the final hi collective, then odd-k chains + a DVE add once it lands.
Host gathers the 8 [512, 2048] row-slices into the (2, 2048, 2048) output.
"""

import os
import sys

sys.path.insert(0, "/opt/trn_rl_repo")

import ml_dtypes
import numpy as np

import concourse.bass as bass
import concourse.mybir as mybir
import concourse.tile as tile
from concourse import bacc
from concourse.bass_utils import run_bass_kernel_spmd
from concourse.masks import make_identity
from concourse.tile_rust import add_dep_helper

N_CORES = 8
B, S, D = 2, 2048, 2048
H, KVH, HD = 32, 4, 64
HPC = H // N_CORES  # 4 q heads per core
ROWS = B * S  # 4096
RPC = S // N_CORES  # 256 output rows per core per batch

F32 = mybir.dt.float32
BF16 = mybir.dt.bfloat16
EXP = mybir.ActivationFunctionType.Exp
ADD = mybir.AluOpType.add
MULT = mybir.AluOpType.mult
DIV = mybir.AluOpType.divide

QKV = 384  # 256 q | 64 k | 64 v
ROPE_W = 320  # rope applies to q + k
VB = 128  # per-chunk block in the PV weights: 64 V | 64 ones


def build():
    nc = bacc.Bacc("TRN2", target_bir_lowering=False, debug=False, num_devices=N_CORES)

    xt = nc.declare_dram_parameter("xt", [D, ROWS], BF16, isOutput=False)
    wqkv = nc.declare_dram_parameter("wqkv", [D, QKV], BF16, isOutput=False)
    wo = nc.declare_dram_parameter("wo", [D, D], BF16, isOutput=False)
    ropec = nc.declare_dram_parameter("ropec", [S, 64], BF16, isOutput=False)
    ropes = nc.declare_dram_parameter("ropes", [S, 64], BF16, isOutput=False)
    maskm = nc.declare_dram_parameter("maskm", [128, 256], BF16, isOutput=False)
    out = nc.declare_dram_parameter("out", [2 * RPC, D], F32, isOutput=True)

    with tile.TileContext(nc) as tc:
        with (
            tc.tile_pool(name="sb", bufs=1) as sb,
            tc.tile_pool(name="ps", bufs=1, space="PSUM") as ps,
            tc.tile_pool(name="dr", bufs=1, space="DRAM") as dr,
        ):
            # ---- constants / weights first so projection starts ASAP ----
            identf = sb.tile([128, 128], F32, tag="identf")
            make_identity(nc, identf[:])
            identb = sb.tile([128, 128], BF16, tag="identb")
            nc.vector.tensor_copy(identb[:], identf[:])
            wqkv_sb = []
            for k in range(16):
                w = sb.tile([128, QKV], BF16, tag=f"wqkv{k}", name=f"wqkv_sb{k}")
                nc.sync.dma_start(out=w[:], in_=wqkv[128 * k : 128 * (k + 1), :])
                wqkv_sb.append(w)
            maskm_sb = sb.tile([128, 256], BF16, tag="maskm")
            # rope tables fully resident in bf16 (loaded just-in-time below)
            ct_all = sb.tile([128, 16 * 64], BF16, tag="ct_all")
            st_all = sb.tile([128, 16 * 64], BF16, tag="st_all")
            # wo tiles are declared here but DMA'd one per phase-1 rowblock so
            # the 8 MB doesn't delay the startup x^T loads.
            wo_sb = [
                sb.tile([128, D], BF16, tag=f"wo{k}", name=f"wo_sb{k}")
                for k in range(16)
            ]

            # ---- persistent per-batch activation tiles ----
            # ktz[b][hi] holds K^T on partition half hi and ZEROS on the other
            # half, so the scores matmul can contract over all 128 partitions
            # (the other head's q rows hit the zero half): every PE instruction
            # stays in 128x128 tile mode - no mode-switch drains.
            qt_sb = [[None, None], [None, None]]  # [b][i]: [128, 2048] bf16
            ktz = [[None, None], [None, None]]  # [b][hi]
            vones = [None, None]  # [b]: [128, 16*VB] bf16 (V | 64 ones columns)
            for b in range(B):
                for i in range(2):
                    t = sb.tile([128, S], BF16, tag=f"qt{b}{i}", name=f"qt{b}{i}")
                    qt_sb[b][i] = t
                    kz = sb.tile([128, S], BF16, tag=f"ktz{b}{i}", name=f"ktz{b}{i}")
                    nc.vector.memset(kz[:], 0.0)
                    ktz[b][i] = kz
                v = sb.tile([128, 16 * VB], BF16, tag=f"v{b}", name=f"vones{b}")
                nc.vector.memset(v[:], 1.0)
                vones[b] = v

            # ================= phase 1: QKV projection + RoPE + transposes
            pend = []  # lagged transpose work
            for rb in range(8):  # 512-row blocks of the flattened (B*S) rows
                xts = []
                for k in range(16):
                    t = sb.tile([128, 512], BF16, tag="xt", bufs=24, name=f"xt_{rb}_{k}")
                    # first block on the (startup-idle) ACT HWDGE queue so it
                    # streams in parallel with the weight loads on Sync; it is
                    # loaded rowtile-major in [128,128] chunks so the very
                    # first matmul chain starts after ~0.5 MB instead of 3.5.
                    if rb > 0:
                        nc.sync.dma_start(
                            out=t[:],
                            in_=xt[128 * k : 128 * (k + 1), 512 * rb : 512 * (rb + 1)],
                        )
                    xts.append(t)
                if rb == 0:
                    for rt0 in range(4):
                        for k in range(16):
                            nc.scalar.dma_start(
                                out=xts[k][:, 128 * rt0 : 128 * (rt0 + 1)],
                                in_=xt[128 * k : 128 * (k + 1), 128 * rt0 : 128 * (rt0 + 1)],
                            )
                if rb == 0:
                    nc.sync.dma_start(out=maskm_sb[:], in_=maskm[:])
                # pace the (phase-4) wo loads through the BACK half of phase 1:
                # the DMA-bound ramp-in gets the bandwidth, and wo still lands
                # before the attention staging needs the sync queue
                if rb >= 4:
                    for w in range(4 * (rb - 4), 4 * (rb - 4) + 4):
                        nc.sync.dma_start(out=wo_sb[w][:], in_=wo[128 * w : 128 * (w + 1), :])
                if rb < 4:  # rope tiles for this block's positions (b1 reuses them)
                    for kc2 in range(4 * rb, 4 * rb + 4):
                        nc.sync.dma_start(
                            out=ct_all[:, 64 * kc2 : 64 * (kc2 + 1)],
                            in_=ropec[128 * kc2 : 128 * (kc2 + 1), :],
                        )
                        nc.sync.dma_start(
                            out=st_all[:, 64 * kc2 : 64 * (kc2 + 1)],
                            in_=ropes[128 * kc2 : 128 * (kc2 + 1), :],
                        )
                for rt in range(4):
                    r = 4 * rb + rt  # global 128-row tile index (0..31)
                    b = r // 16
                    kc = r % 16  # position tile within the batch
                    pq = ps.tile([128, QKV], F32, tag="pq", bufs=2, name=f"pq_{r}")
                    for k in range(16):
                        nc.tensor.matmul(
                            pq[:],
                            xts[k][:, 128 * rt : 128 * (rt + 1)],
                            wqkv_sb[k][:],
                            start=(k == 0),
                            stop=(k == 15),
                        )
                    # single psum read frees the pq slot in ~0.6us; RoPE and
                    # the V copy then work from SBUF (bf16 fast modes)
                    pqc = sb.tile([128, QKV], BF16, tag="pqc", bufs=2, name=f"pc_{r}")
                    nc.vector.tensor_copy(pqc[:], pq[:])
                    # RoPE over q + k (5 head-blocks of [32r|32i]); the cos/sin
                    # tables hold ONE 64-col block per position chunk, broadcast
                    # across the 5 head-blocks with a stride-0 AP dim
                    ct = ct_all[:, 64 * kc : 64 * (kc + 1)].unsqueeze(1).broadcast_to([128, 5, 64])
                    st = st_all[:, 64 * kc : 64 * (kc + 1)]
                    stv = st.rearrange("p (s j) -> p s j", j=32).unsqueeze(1).broadcast_to([128, 5, 2, 32])
                    tmp1 = sb.tile([128, ROPE_W], BF16, tag="tmp1", bufs=2, name=f"t1_{r}")
                    tmp2 = sb.tile([128, ROPE_W], BF16, tag="tmp2", bufs=2, name=f"t2_{r}")
                    qk = sb.tile([128, ROPE_W], BF16, tag="qk", bufs=4, name=f"qk_{r}")
                    nc.vector.tensor_tensor(
                        tmp1[:].rearrange("p (h j) -> p h j", j=64),
                        pqc[:, 0:ROPE_W].rearrange("p (h j) -> p h j", j=64),
                        ct, op=MULT,
                    )
                    pqv = pqc[:, 0:ROPE_W].rearrange("p (h s j) -> p h s j", s=2, j=32)
                    t2v = tmp2[:].rearrange("p (h s j) -> p h s j", s=2, j=32)
                    # out real-half = q_imag * (-sin); out imag-half = q_real * (+sin)
                    nc.vector.tensor_tensor(
                        t2v[:, :, 0, :], pqv[:, :, 1, :], stv[:, :, 0, :], op=MULT
                    )
                    nc.vector.tensor_tensor(
                        t2v[:, :, 1, :], pqv[:, :, 0, :], stv[:, :, 1, :], op=MULT
                    )
                    nc.vector.tensor_tensor(qk[:], tmp1[:], tmp2[:], op=ADD)
                    # V -> bf16 into the ones-padded PV weights, on the (idle
                    # until the collectives) gpsimd so ACT stays exp-only
                    nc.gpsimd.tensor_copy(vones[b][:, VB * kc : VB * kc + 64], pqc[:, 320:384])
                    # PE transposes, lagged one rowtile so the RoPE chain has
                    # a full projection's lead time
                    pend.append((qk, b, kc, r))
                    todo = [pend.pop(0)] if len(pend) > 1 else []
                    if r == 31:
                        todo += [pend.pop(0)]
                    for tqk, tb, tkc, tr in todo:
                        for i in range(2):
                            tp = ps.tile(
                                [128, 128], BF16, tag="pq", bufs=2, name=f"tp_{tr}_{i}"
                            )
                            nc.tensor.transpose(
                                tp[:], tqk[:, 128 * i : 128 * (i + 1)], identb[:]
                            )
                            nc.vector.tensor_copy(
                                qt_sb[tb][i][:, 128 * tkc : 128 * (tkc + 1)], tp[:]
                            )
                        # K: [128, 64] -> [64, 128], then copy into the live
                        # half of each zero-padded kt variant
                        tpk = ps.tile([64, 128], BF16, tag="pq", bufs=2, name=f"tpk_{tr}")
                        nc.tensor.transpose(tpk[:], tqk[:, 256:320], identb[:])
                        nc.vector.tensor_copy(
                            ktz[tb][0][0:64, 128 * tkc : 128 * (tkc + 1)], tpk[:]
                        )
                        nc.vector.tensor_copy(
                            ktz[tb][1][64:128, 128 * tkc : 128 * (tkc + 1)], tpk[:]
                        )

            # ================= phases 2+3: attention per batch, then AllToAll
            # run attention strictly after phase 1 (the overlap costs more in
            # in-order-queue stalls than it saves)
            a2a_out = [[None, None], [None, None]]  # [b][half]
            last_pv = None  # ordering handles for the output projection
            pin_a = None  # oproj phase A starts after batch-1's first head-pair
            ats = [
                sb.tile([128, 512], BF16, tag=f"at{k}", name=f"at_{k}") for k in range(16)
            ]
            for b in range(B):
                a2a_in = [
                    dr.tile([1024, RPC], BF16, tag=f"a2ai{b}{p}", name=f"a2a_in{b}{p}")
                    for p in range(2)
                ]
                a2a_out[b] = [
                    dr.tile([1024, RPC], BF16, tag=f"a2ao{b}{p}", name=f"a2a_out{b}{p}")
                    for p in range(2)
                ]
                for hp in range(2):  # head pair (2hp, 2hp+1): one PE row-tile each
                    qtile = qt_sb[b][hp]
                    for qc in range(4):  # 512-wide q chunks
                        ots = [
                            ps.tile([128, 512], F32, tag=f"ot{hi}", bufs=1,
                                    name=f"ot_{b}_{hp}_{qc}_{hi}")
                            for hi in range(2)
                        ]
                        # kpos chunk groups: two packed diagonal groups first
                        # (columns < the chunk's causal start are dropped), then
                        # clean pairs descending. Each chunk is (kch, colstart,
                        # width): scores/exp/PV only touch cols [cs, 512).
                        groups = [
                            [(4 * qc, 0, 512), (4 * qc + 1, 128, 384)],
                            [(4 * qc + 2, 256, 256), (4 * qc + 3, 384, 128)],
                        ] + [
                            [(2 * p, 0, 512), (2 * p + 1, 0, 512)]
                            for p in reversed(range(2 * qc))
                        ]
                        pend_pv = None
                        for gi, chunks in enumerate(groups + [None]):
                            pts = []
                            if chunks is not None:
                                diag = gi < 2
                                for hi in range(2):
                                    sp = ps.tile(
                                        [128, 1024], F32, tag="s", bufs=2,
                                        name=f"s_{b}_{hp}_{qc}_{gi}_{hi}",
                                    )
                                    off, offs = 0, []
                                    for kch, cs, w in chunks:
                                        nc.tensor.matmul(
                                            sp[:, off : off + w],
                                            ktz[b][hi][:, 128 * kch : 128 * (kch + 1)],
                                            qtile[:, 512 * qc + cs : 512 * qc + cs + w],
                                            start=True,
                                            stop=True,
                                        )
                                        offs.append(off)
                                        off += w
                                    pt = sb.tile(
                                        [128, 1024], BF16, tag="pt", bufs=6,
                                        name=f"pt_{b}_{hp}_{qc}_{gi}_{hi}",
                                    )
                                    nc.scalar.activation(pt[:, 0:off], sp[:, 0:off], EXP, scale=0.125)
                                    if diag:
                                        # per-chunk leading 128 cols are the causal
                                        # triangle: one strided 0/1 multiply covers
                                        # both chunks of the group
                                        stride = 512 if gi == 0 else 256
                                        ptv = pt[:, 0 : 2 * stride].rearrange(
                                            "p (g c) -> p g c", c=stride
                                        )[:, :, 0:128]
                                        mkv = maskm_sb[:].rearrange("p (g c) -> p g c", c=128)
                                        nc.vector.tensor_tensor(ptv, ptv, mkv, op=MULT)
                                    pts.append((pt, chunks, offs))
                            if pend_pv is not None:
                                for hi in range(2):
                                    pt, chunks2, offs2 = pend_pv[hi]
                                    for (kch, cs, w), po in zip(chunks2, offs2):
                                        mm = nc.tensor.matmul(
                                            ots[hi][:, cs : cs + w],
                                            vones[b][:, VB * kch : VB * kch + VB],
                                            pt[:, po : po + w],
                                            start=(kch == 4 * qc),
                                            stop=(kch == (1 if qc else 3)),
                                            skip_group_check=True,
                                        )
                                        last_pv = mm
                            pend_pv = pts if chunks is not None else None
                        # normalize: rows 0:64 are V^T P, rows 64:128 the softmax
                        # sums (64 identical copies): copy sums to base-0 SBUF
                        # (custom-DVE recip needs aligned bases), reciprocal,
                        # one 64-lane multiply straight from psum.
                        for hi in range(2):
                            h = 2 * hp + hi
                            ot = ots[hi]
                            sums = sb.tile([64, 512], F32, tag="sums", bufs=2, name=f"sm_{b}_{h}_{qc}")
                            nc.vector.tensor_copy(sums[:], ot[64:128, :])
                            inv = sb.tile([64, 512], F32, tag="inv", bufs=2, name=f"iv_{b}_{h}_{qc}")
                            nc.vector.reciprocal_approx_fast(inv[:], sums[:])
                            osb = sb.tile([64, 512], BF16, tag="osb", bufs=3, name=f"o_{b}_{h}_{qc}")
                            nc.vector.tensor_tensor(osb[:], ot[0:64, :], inv[:], op=MULT)
                            # stage into AllToAll layout: dest j rows 128j..128j+128
                            # in the lo (heads 0-1) / hi (heads 2-3) half buffer
                            for half in range(2):
                                j = 2 * qc + half
                                nc.sync.dma_start(
                                    out=a2a_in[hp][128 * j + 64 * hi : 128 * j + 64 * (hi + 1), :],
                                    in_=osb[:, 256 * half : 256 * (half + 1)],
                                )
                    # lo half-collective after heads 0-1, hi after heads 2-3
                    nc.gpsimd.collective_compute(
                        "AllToAll",
                        mybir.AluOpType.bypass,
                        replica_groups=[list(range(N_CORES))],
                        ins=[a2a_in[hp][:].opt()],
                        outs=[a2a_out[b][hp][:].opt()],
                    )
                    # at-tile loads right after each half-collective, on the
                    # gpsimd queue (so Sync/PE never block on a collective);
                    # the last batch's hi loads split with the idle ACT
                    # queue to halve the issue tail gating oproj phase B.
                    # Even k-tiles come from lo, odd from hi.
                    for k in range(hp, 16, 2):
                        eng = nc.scalar if (b == 1 and hp == 1 and k % 4 == 3) else nc.gpsimd
                        eng.dma_start(
                            out=ats[k][:, 256 * b : 256 * (b + 1)],
                            in_=a2a_out[b][hp][128 * (k // 2) : 128 * (k // 2) + 128, :],
                        )
                    if b == 1 and hp == 0:
                        pin_a = last_pv

            # ================= phase 4: output projection (my 512 rows @ wo)
            # phase A: batch-0 rows (need only batch-0 collectives), pinned
            # only after batch-1's FIRST head-pair so its matmuls fill the PE
            # slack of the ACT-bound second half. Phase B (batch-1 rows) runs
            # as even-k partial chains (drained to SBUF) that need only the lo
            # half-collective - they execute WHILE the final hi collective is
            # in flight - then odd-k chains + a DVE add once it lands.
            korder = list(range(0, 16, 2)) + list(range(1, 16, 2))
            prev_phase_last = pin_a if pin_a is not None else last_pv
            phase_last = None
            for n in range(4):
                for row in (0, 1):
                    op = ps.tile([128, 512], F32, tag="pq", bufs=2, name=f"op_{n}_{row}")
                    for ki, k in enumerate(korder):
                        mm = nc.tensor.matmul(
                            op[:],
                            ats[k][:, 128 * row : 128 * (row + 1)],
                            wo_sb[k][:, 512 * n : 512 * (n + 1)],
                            start=(ki == 0),
                            stop=(ki == 15),
                        )
                        if ki == 0 and prev_phase_last is not None:
                            add_dep_helper(
                                mm.ins,
                                prev_phase_last.ins,
                                sync=False,
                                reason="pin oproj phase order in PE queue",
                            )
                        phase_last = mm
                    ob = sb.tile([128, 512], F32, tag="outsb", bufs=2, name=f"ob_{n}_{row}")
                    nc.vector.tensor_copy(ob[:], op[:])
                    nc.sync.dma_start(
                        out=out[128 * row : 128 * (row + 1), 512 * n : 512 * (n + 1)],
                        in_=ob[:],
                    )
            prev_phase_last = phase_last
            partials = {}
            for n in range(4):
                for row in (2, 3):
                    op = ps.tile([128, 512], F32, tag="pq", bufs=2, name=f"ope_{n}_{row}")
                    for ki, k in enumerate(korder[:8]):
                        mm = nc.tensor.matmul(
                            op[:],
                            ats[k][:, 128 * row : 128 * (row + 1)],
                            wo_sb[k][:, 512 * n : 512 * (n + 1)],
                            start=(ki == 0),
                            stop=(ki == 7),
                        )
                        if ki == 0 and prev_phase_last is not None:
                            add_dep_helper(
                                mm.ins,
                                prev_phase_last.ins,
                                sync=False,
                                reason="pin oproj phase order in PE queue",
                            )
                        prev_phase_last = mm
                    pe_sb = sb.tile([128, 512], BF16, tag="pesb", bufs=8, name=f"pe_{n}_{row}")
                    nc.vector.tensor_copy(pe_sb[:], op[:])
                    partials[(n, row)] = pe_sb
            for n in range(4):
                for row in (2, 3):
                    op = ps.tile([128, 512], F32, tag="pq", bufs=2, name=f"opo_{n}_{row}")
                    for ki, k in enumerate(korder[8:]):
                        mm = nc.tensor.matmul(
                            op[:],
                            ats[k][:, 128 * row : 128 * (row + 1)],
                            wo_sb[k][:, 512 * n : 512 * (n + 1)],
                            start=(ki == 0),
                            stop=(ki == 7),
                        )
                        prev_phase_last = mm
                    ob = sb.tile([128, 512], F32, tag="outsb", bufs=2, name=f"ob_{n}_{row}")
                    nc.vector.tensor_tensor(ob[:], op[:], partials[(n, row)][:], op=ADD)
                    nc.sync.dma_start(
                        out=out[128 * row : 128 * (row + 1), 512 * n : 512 * (n + 1)],
                        in_=ob[:],
                    )

    nc.finalize()
    return nc


_NC_CACHE = None


def _get_nc():
    global _NC_CACHE
    if _NC_CACHE is None:
        _NC_CACHE = build()
    return _NC_CACHE


def _prep_inputs(x, freqs_cis, mask, wq, wk, wv, wo):
    """Host-side sharding / layout prep. Returns per-core input maps."""
    bf16 = ml_dtypes.bfloat16
    xt = np.ascontiguousarray(x.reshape(ROWS, D).T.astype(bf16))  # [D, B*S]
    cos = np.ascontiguousarray(freqs_cis[:, :, 0])  # [S, 32]
    sin = np.ascontiguousarray(freqs_cis[:, :, 1])
    c64 = np.concatenate([cos, cos], axis=1)  # [S, 64]
    s64 = np.concatenate([-sin, sin], axis=1)
    ropec = np.ascontiguousarray(c64.astype(bf16))  # [S, 64]
    ropes = np.ascontiguousarray(s64.astype(bf16))
    # causal 0/1 keep-triangle (keep iff q-col >= kpos-row), replicated twice so
    # one strided multiply masks both chunks of a diagonal group. Derived from
    # the mask input: maskm[r, c] = keep(mask[c, r]) for the leading 128x128.
    tri = (mask[0:128, 0:128].T > -1.0).astype(bf16)
    maskm = np.ascontiguousarray(np.concatenate([tri, tri], axis=1))
    perm = np.concatenate([np.arange(0, 64, 2), np.arange(1, 64, 2)])  # de-interleave
    wo_c = np.ascontiguousarray(wo.astype(bf16))

    in_maps = []
    for c in range(N_CORES):
        heads = range(HPC * c, HPC * (c + 1))
        kv = c // 2
        wq_c = np.concatenate([wq[:, 64 * h + perm] for h in heads], axis=1)
        wk_c = wk[:, 64 * kv + perm]
        wv_c = wv[:, 64 * kv : 64 * (kv + 1)]
        wqkv_c = np.ascontiguousarray(
            np.concatenate([wq_c, wk_c, wv_c], axis=1).astype(bf16)
        )
        in_maps.append(
            {
                "xt": xt,
                "wqkv": wqkv_c,
                "wo": wo_c,
                "ropec": ropec,
                "ropes": ropes,
                "maskm": maskm,
            }
        )
    return in_maps


def kernel(x, freqs_cis, mask, wq, wk, wv, wo, _trace=False, _trace_kwargs=None):
    nc = _get_nc()
    in_maps = _prep_inputs(
        np.asarray(x, np.float32),
        np.asarray(freqs_cis, np.float32),
        np.asarray(mask, np.float32),
        np.asarray(wq, np.float32),
        np.asarray(wk, np.float32),
        np.asarray(wv, np.float32),
        np.asarray(wo, np.float32),
    )
    kwargs = {}
    if _trace:
        kwargs["trace"] = True
        if _trace_kwargs:
            kwargs.update(_trace_kwargs)
    res = run_bass_kernel_spmd(nc, in_maps, core_ids=list(range(N_CORES)), **kwargs)
    full = np.empty((B, S, D), np.float32)
    for c in range(N_CORES):
        oc = res.results[c]["out"]
        full[0, RPC * c : RPC * (c + 1)] = oc[0:RPC]
        full[1, RPC * c : RPC * (c + 1)] = oc[RPC : 2 * RPC]
    if _trace:
        kernel.last_results = res
    return full


if __name__ == "__main__":
    print("building...")
    nc = _get_nc()
    print("built")


# revision 35
# speedup vs baseline: 1.0162x; 1.0162x over previous
"""Distributed GQA attention kernel for 8 TRN2 NeuronCores.

Problem: B=2, S=2048, D=2048, H=32 heads, KVH=4 kv-heads, HD=64 (GQA),
RoPE + causal attention + output projection, fp32 inputs/outputs.

Sharding: tensor-parallel over heads. Core c owns q-heads [4c..4c+4) and
kv-head c//2 (each kv head is shared by 2 cores; its tiny K/V projection is
recomputed on both). Per core:
  1. QKV projection from the replicated, host-pre-transposed x^T (bf16) with
     the core's weight column slice packed as one [2048, 384] bf16 rhs
     (256 q | 64 k | 64 v).
  2. RoPE in natural layout on the DVE (weight columns de-interleaved on host
     so each head is [32 reals | 32 imags]; q.k is invariant under a common
     permutation of head dims).
  3. Q,K transposed on the PE; K's [64,128] transpose lands in TWO
     zero-padded kt variants (K^T on one partition half, zeros on the other)
     so scores contract over all 128 partitions of the two-head q tile and
     every PE instruction stays in 128x128 tile mode (no mode-switch drains).
     Scores are computed transposed (scoresT[kpos, q]) so the softmax
     normalizer falls out of ones-columns appended to V in the PV matmul.
  4. Causal flash attention in bf16, head pairs interleaved. Per q chunk:
     two packed diagonal groups (fully-masked leading columns of each kpos
     chunk dropped from scores/exp/PV; one strided 0/1 triangle multiply per
     group) run first, then clean kpos-chunk pairs descending, software-
     pipelined one group deep so exps overlap the next group's scores.
  5. Normalization entirely on DVE: 64 replicated ones-columns in the PV
     weights land the sums in psum rows 64:128; copy to base-0 SBUF (custom-
     DVE recip requires aligned bases), 64-lane reciprocal, one multiply
     straight from psum. (gpsimd carries ONLY collectives + at-tile loads,
     so batch-1 compute never queues behind the batch-0 AllToAll.)
  6. Attention outputs staged (transposed) to DRAM in AllToAll layout; TWO
     half-collectives per batch (heads 0-1, then heads 2-3) so comm starts
     halfway through each batch's attention and the final collective only
     carries 0.5 MB.
  7. Row-sharded output projection (rows [256c..256c+256) of each batch)
     against the fully-resident bf16 wo. Batch-0 rows are pinned only after
     batch-1's first head-pair so they fill the ACT-bound second half;
     batch-1 rows run as even-k partial chains (drained to SBUF bf16) DURING
     the final hi collective, then odd-k chains + a DVE add once it lands.
Host gathers the 8 [512, 2048] row-slices into the (2, 2048, 2048) output.
"""

import os
import sys

sys.path.insert(0, "/opt/trn_rl_repo")

import ml_dtypes
import numpy as np

import concourse.bass as bass
import concourse.mybir as mybir
import concourse.tile as tile
from concourse import bacc
from concourse.bass_utils import run_bass_kernel_spmd
from concourse.masks import make_identity
from concourse.tile_rust import add_dep_helper

N_CORES = 8
B, S, D = 2, 2048, 2048
H, KVH, HD = 32, 4, 64
HPC = H // N_CORES  # 4 q heads per core
ROWS = B * S  # 4096
RPC = S // N_CORES  # 256 output rows per core per batch

F32 = mybir.dt.float32
BF16 = mybir.dt.bfloat16
EXP = mybir.ActivationFunctionType.Exp
ADD = mybir.AluOpType.add
MULT = mybir.AluOpType.mult
DIV = mybir.AluOpType.divide

QKV = 384  # 256 q | 64 k | 64 v
ROPE_W = 320  # rope applies to q + k
VB = 128  # per-chunk block in the PV weights: 64 V | 64 ones


def build():
    nc = bacc.Bacc("TRN2", target_bir_lowering=False, debug=False, num_devices=N_CORES)

    xt = nc.declare_dram_parameter("xt", [D, ROWS], BF16, isOutput=False)
    wqkv = nc.declare_dram_parameter("wqkv", [D, QKV], BF16, isOutput=False)
    wo = nc.declare_dram_parameter("wo", [D, D], BF16, isOutput=False)
    ropec = nc.declare_dram_parameter("ropec", [S, 64], BF16, isOutput=False)
    ropes = nc.declare_dram_parameter("ropes", [S, 64], BF16, isOutput=False)
    maskm = nc.declare_dram_parameter("maskm", [128, 256], BF16, isOutput=False)
    out = nc.declare_dram_parameter("out", [2 * RPC, D], F32, isOutput=True)

    with tile.TileContext(nc) as tc:
        with (
            tc.tile_pool(name="sb", bufs=1) as sb,
            tc.tile_pool(name="ps", bufs=1, space="PSUM") as ps,
            tc.tile_pool(name="dr", bufs=1, space="DRAM") as dr,
        ):
            # ---- constants / weights first so projection starts ASAP ----
            identf = sb.tile([128, 128], F32, tag="identf")
            make_identity(nc, identf[:])
            identb = sb.tile([128, 128], BF16, tag="identb")
            nc.vector.tensor_copy(identb[:], identf[:])
            wqkv_sb = []
            for k in range(16):
                w = sb.tile([128, QKV], BF16, tag=f"wqkv{k}", name=f"wqkv_sb{k}")
                nc.sync.dma_start(out=w[:], in_=wqkv[128 * k : 128 * (k + 1), :])
                wqkv_sb.append(w)
            maskm_sb = sb.tile([128, 256], BF16, tag="maskm")
            # rope tables fully resident in bf16 (loaded just-in-time below)
            ct_all = sb.tile([128, 16 * 64], BF16, tag="ct_all")
            st_all = sb.tile([128, 16 * 64], BF16, tag="st_all")
            # wo tiles are declared here but DMA'd one per phase-1 rowblock so
            # the 8 MB doesn't delay the startup x^T loads.
            wo_sb = [
                sb.tile([128, D], BF16, tag=f"wo{k}", name=f"wo_sb{k}")
                for k in range(16)
            ]

            # ---- persistent per-batch activation tiles ----
            # ktz[b][hi] holds K^T on partition half hi and ZEROS on the other
            # half, so the scores matmul can contract over all 128 partitions
            # (the other head's q rows hit the zero half): every PE instruction
            # stays in 128x128 tile mode - no mode-switch drains.
            qt_sb = [[None, None], [None, None]]  # [b][i]: [128, 2048] bf16
            ktz = [[None, None], [None, None]]  # [b][hi]
            vones = [None, None]  # [b]: [128, 16*VB] bf16 (V | 64 ones columns)
            for b in range(B):
                for i in range(2):
                    t = sb.tile([128, S], BF16, tag=f"qt{b}{i}", name=f"qt{b}{i}")
                    qt_sb[b][i] = t
                    kz = sb.tile([128, S], BF16, tag=f"ktz{b}{i}", name=f"ktz{b}{i}")
                    nc.vector.memset(kz[:], 0.0)
                    ktz[b][i] = kz
                v = sb.tile([128, 16 * VB], BF16, tag=f"v{b}", name=f"vones{b}")
                nc.vector.memset(v[:], 1.0)
                vones[b] = v

            # ================= phase 1: QKV projection + RoPE + transposes
            pend = []  # lagged transpose work
            for rb in range(8):  # 512-row blocks of the flattened (B*S) rows
                xts = []
                for k in range(16):
                    t = sb.tile([128, 512], BF16, tag="xt", bufs=18, name=f"xt_{rb}_{k}")
                    # first block on the (startup-idle) ACT HWDGE queue so it
                    # streams in parallel with the weight loads on Sync; it is
                    # loaded rowtile-major in [128,128] chunks so the very
                    # first matmul chain starts after ~0.5 MB instead of 3.5.
                    if rb > 0:
                        nc.sync.dma_start(
                            out=t[:],
                            in_=xt[128 * k : 128 * (k + 1), 512 * rb : 512 * (rb + 1)],
                        )
                    xts.append(t)
                if rb == 0:
                    for rt0 in range(4):
                        for k in range(16):
                            nc.scalar.dma_start(
                                out=xts[k][:, 128 * rt0 : 128 * (rt0 + 1)],
                                in_=xt[128 * k : 128 * (k + 1), 128 * rt0 : 128 * (rt0 + 1)],
                            )
                if rb == 0:
                    nc.sync.dma_start(out=maskm_sb[:], in_=maskm[:])
                # pace the (phase-4) wo loads through the BACK half of phase 1:
                # the DMA-bound ramp-in gets the bandwidth, and wo still lands
                # before the attention staging needs the sync queue
                if rb >= 4:
                    for w in range(4 * (rb - 4), 4 * (rb - 4) + 4):
                        nc.sync.dma_start(out=wo_sb[w][:], in_=wo[128 * w : 128 * (w + 1), :])
                if rb < 4:  # rope tiles for this block's positions (b1 reuses them)
                    for kc2 in range(4 * rb, 4 * rb + 4):
                        nc.sync.dma_start(
                            out=ct_all[:, 64 * kc2 : 64 * (kc2 + 1)],
                            in_=ropec[128 * kc2 : 128 * (kc2 + 1), :],
                        )
                        nc.sync.dma_start(
                            out=st_all[:, 64 * kc2 : 64 * (kc2 + 1)],
                            in_=ropes[128 * kc2 : 128 * (kc2 + 1), :],
                        )
                for rt in range(4):
                    r = 4 * rb + rt  # global 128-row tile index (0..31)
                    b = r // 16
                    kc = r % 16  # position tile within the batch
                    pq = ps.tile([128, QKV], F32, tag="pq", bufs=2, name=f"pq_{r}")
                    for k in range(16):
                        nc.tensor.matmul(
                            pq[:],
                            xts[k][:, 128 * rt : 128 * (rt + 1)],
                            wqkv_sb[k][:],
                            start=(k == 0),
                            stop=(k == 15),
                        )
                    # single psum read frees the pq slot in ~0.6us; RoPE and
                    # the V copy then work from SBUF (bf16 fast modes)
                    pqc = sb.tile([128, QKV], BF16, tag="pqc", bufs=2, name=f"pc_{r}")
                    nc.vector.tensor_copy(pqc[:], pq[:])
                    # RoPE over q + k (5 head-blocks of [32r|32i]); the cos/sin
                    # tables hold ONE 64-col block per position chunk, broadcast
                    # across the 5 head-blocks with a stride-0 AP dim
                    ct = ct_all[:, 64 * kc : 64 * (kc + 1)].unsqueeze(1).broadcast_to([128, 5, 64])
                    st = st_all[:, 64 * kc : 64 * (kc + 1)]
                    stv = st.rearrange("p (s j) -> p s j", j=32).unsqueeze(1).broadcast_to([128, 5, 2, 32])
                    tmp1 = sb.tile([128, ROPE_W], BF16, tag="tmp1", bufs=2, name=f"t1_{r}")
                    tmp2 = sb.tile([128, ROPE_W], BF16, tag="tmp2", bufs=2, name=f"t2_{r}")
                    qk = sb.tile([128, ROPE_W], BF16, tag="qk", bufs=4, name=f"qk_{r}")
                    nc.vector.tensor_tensor(
                        tmp1[:].rearrange("p (h j) -> p h j", j=64),
                        pqc[:, 0:ROPE_W].rearrange("p (h j) -> p h j", j=64),
                        ct, op=MULT,
                    )
                    pqv = pqc[:, 0:ROPE_W].rearrange("p (h s j) -> p h s j", s=2, j=32)
                    t2v = tmp2[:].rearrange("p (h s j) -> p h s j", s=2, j=32)
                    # out real-half = q_imag * (-sin); out imag-half = q_real * (+sin)
                    nc.vector.tensor_tensor(
                        t2v[:, :, 0, :], pqv[:, :, 1, :], stv[:, :, 0, :], op=MULT
                    )
                    nc.vector.tensor_tensor(
                        t2v[:, :, 1, :], pqv[:, :, 0, :], stv[:, :, 1, :], op=MULT
                    )
                    nc.vector.tensor_tensor(qk[:], tmp1[:], tmp2[:], op=ADD)
                    # V -> bf16 into the ones-padded PV weights, on the (idle
                    # until the collectives) gpsimd so ACT stays exp-only
                    nc.gpsimd.tensor_copy(vones[b][:, VB * kc : VB * kc + 64], pqc[:, 320:384])
                    # PE transposes, lagged one rowtile so the RoPE chain has
                    # a full projection's lead time
                    pend.append((qk, b, kc, r))
                    todo = [pend.pop(0)] if len(pend) > 1 else []
                    if r == 31:
                        todo += [pend.pop(0)]
                    for tqk, tb, tkc, tr in todo:
                        for i in range(2):
                            tp = ps.tile(
                                [128, 128], BF16, tag="pq", bufs=2, name=f"tp_{tr}_{i}"
                            )
                            nc.tensor.transpose(
                                tp[:], tqk[:, 128 * i : 128 * (i + 1)], identb[:]
                            )
                            nc.vector.tensor_copy(
                                qt_sb[tb][i][:, 128 * tkc : 128 * (tkc + 1)], tp[:]
                            )
                        # K: [128, 64] -> [64, 128], then copy into the live
                        # half of each zero-padded kt variant
                        tpk = ps.tile([64, 128], BF16, tag="pq", bufs=2, name=f"tpk_{tr}")
                        nc.tensor.transpose(tpk[:], tqk[:, 256:320], identb[:])
                        nc.vector.tensor_copy(
                            ktz[tb][0][0:64, 128 * tkc : 128 * (tkc + 1)], tpk[:]
                        )
                        nc.vector.tensor_copy(
                            ktz[tb][1][64:128, 128 * tkc : 128 * (tkc + 1)], tpk[:]
                        )

            # ================= phases 2+3: attention per batch, then AllToAll
            # run attention strictly after phase 1 (the overlap costs more in
            # in-order-queue stalls than it saves)
            a2a_out = [[None, None], [None, None]]  # [b][half]
            last_pv = None  # ordering handles for the output projection
            pin_a = None  # oproj phase A starts after batch-1's first head-pair
            ats = [
                sb.tile([128, 512], BF16, tag=f"at{k}", name=f"at_{k}") for k in range(16)
            ]
            for b in range(B):
                a2a_in = [
                    dr.tile([1024, RPC], BF16, tag=f"a2ai{b}{p}", name=f"a2a_in{b}{p}")
                    for p in range(2)
                ]
                a2a_out[b] = [
                    dr.tile([1024, RPC], BF16, tag=f"a2ao{b}{p}", name=f"a2a_out{b}{p}")
                    for p in range(2)
                ]
                for hp in range(2):  # head pair (2hp, 2hp+1): one PE row-tile each
                    qtile = qt_sb[b][hp]
                    for qc in range(4):  # 512-wide q chunks
                        ots = [
                            ps.tile([128, 512], F32, tag=f"ot{hi}", bufs=1,
                                    name=f"ot_{b}_{hp}_{qc}_{hi}")
                            for hi in range(2)
                        ]
                        # kpos chunk groups: two packed diagonal groups first
                        # (columns < the chunk's causal start are dropped), then
                        # clean pairs descending. Each chunk is (kch, colstart,
                        # width): scores/exp/PV only touch cols [cs, 512).
                        groups = [
                            [(4 * qc, 0, 512), (4 * qc + 1, 128, 384)],
                            [(4 * qc + 2, 256, 256), (4 * qc + 3, 384, 128)],
                        ] + [
                            [(2 * p, 0, 512), (2 * p + 1, 0, 512)]
                            for p in reversed(range(2 * qc))
                        ]
                        pend_pv = None
                        for gi, chunks in enumerate(groups + [None]):
                            pts = []
                            if chunks is not None:
                                diag = gi < 2
                                for hi in range(2):
                                    sp = ps.tile(
                                        [128, 1024], F32, tag="s", bufs=2,
                                        name=f"s_{b}_{hp}_{qc}_{gi}_{hi}",
                                    )
                                    off, offs = 0, []
                                    for kch, cs, w in chunks:
                                        nc.tensor.matmul(
                                            sp[:, off : off + w],
                                            ktz[b][hi][:, 128 * kch : 128 * (kch + 1)],
                                            qtile[:, 512 * qc + cs : 512 * qc + cs + w],
                                            start=True,
                                            stop=True,
                                        )
                                        offs.append(off)
                                        off += w
                                    pt = sb.tile(
                                        [128, 1024], BF16, tag="pt", bufs=6,
                                        name=f"pt_{b}_{hp}_{qc}_{gi}_{hi}",
                                    )
                                    nc.scalar.activation(pt[:, 0:off], sp[:, 0:off], EXP, scale=0.125)
                                    if diag:
                                        # per-chunk leading 128 cols are the causal
                                        # triangle: one strided 0/1 multiply covers
                                        # both chunks of the group
                                        stride = 512 if gi == 0 else 256
                                        ptv = pt[:, 0 : 2 * stride].rearrange(
                                            "p (g c) -> p g c", c=stride
                                        )[:, :, 0:128]
                                        mkv = maskm_sb[:].rearrange("p (g c) -> p g c", c=128)
                                        nc.vector.tensor_tensor(ptv, ptv, mkv, op=MULT)
                                    pts.append((pt, chunks, offs))
                            if pend_pv is not None:
                                for hi in range(2):
                                    pt, chunks2, offs2 = pend_pv[hi]
                                    for (kch, cs, w), po in zip(chunks2, offs2):
                                        mm = nc.tensor.matmul(
                                            ots[hi][:, cs : cs + w],
                                            vones[b][:, VB * kch : VB * kch + VB],
                                            pt[:, po : po + w],
                                            start=(kch == 4 * qc),
                                            stop=(kch == (1 if qc else 3)),
                                            skip_group_check=True,
                                        )
                                        last_pv = mm
                            pend_pv = pts if chunks is not None else None
                        # normalize: rows 0:64 are V^T P, rows 64:128 the softmax
                        # sums (64 identical copies): copy sums to base-0 SBUF
                        # (custom-DVE recip needs aligned bases), reciprocal,
                        # one 64-lane multiply straight from psum.
                        for hi in range(2):
                            h = 2 * hp + hi
                            ot = ots[hi]
                            sums = sb.tile([64, 512], F32, tag="sums", bufs=2, name=f"sm_{b}_{h}_{qc}")
                            nc.vector.tensor_copy(sums[:], ot[64:128, :])
                            inv = sb.tile([64, 512], F32, tag="inv", bufs=2, name=f"iv_{b}_{h}_{qc}")
                            nc.vector.reciprocal_approx_fast(inv[:], sums[:])
                            osb = sb.tile([64, 512], BF16, tag="osb", bufs=3, name=f"o_{b}_{h}_{qc}")
                            nc.vector.tensor_tensor(osb[:], ot[0:64, :], inv[:], op=MULT)
                            # stage into AllToAll layout: dest j rows 128j..128j+128
                            # in the lo (heads 0-1) / hi (heads 2-3) half buffer
                            for half in range(2):
                                j = 2 * qc + half
                                nc.sync.dma_start(
                                    out=a2a_in[hp][128 * j + 64 * hi : 128 * j + 64 * (hi + 1), :],
                                    in_=osb[:, 256 * half : 256 * (half + 1)],
                                )
                    # lo half-collective after heads 0-1, hi after heads 2-3
                    nc.gpsimd.collective_compute(
                        "AllToAll",
                        mybir.AluOpType.bypass,
                        replica_groups=[list(range(N_CORES))],
                        ins=[a2a_in[hp][:].opt()],
                        outs=[a2a_out[b][hp][:].opt()],
                    )
                    # at-tile loads right after each half-collective, on the
                    # gpsimd queue (so Sync/PE never block on a collective);
                    # the last batch's hi loads split with the idle ACT
                    # queue to halve the issue tail gating oproj phase B.
                    # Even k-tiles come from lo, odd from hi.
                    for k in range(hp, 16, 2):
                        eng = nc.scalar if (b == 1 and hp == 1 and k % 4 == 3) else nc.gpsimd
                        eng.dma_start(
                            out=ats[k][:, 256 * b : 256 * (b + 1)],
                            in_=a2a_out[b][hp][128 * (k // 2) : 128 * (k // 2) + 128, :],
                        )
                    if b == 1 and hp == 0:
                        pin_a = last_pv

            # ================= phase 4: output projection (my 512 rows @ wo)
            # phase A: batch-0 rows (need only batch-0 collectives), pinned
            # only after batch-1's FIRST head-pair so its matmuls fill the PE
            # slack of the ACT-bound second half. Phase B (batch-1 rows) runs
            # as even-k partial chains (drained to SBUF) that need only the lo
            # half-collective - they execute WHILE the final hi collective is
            # in flight - then odd-k chains + a DVE add once it lands.
            korder = list(range(0, 16, 2)) + list(range(1, 16, 2))
            prev_phase_last = pin_a if pin_a is not None else last_pv
            phase_last = None
            for n in range(4):
                for row in (0, 1):
                    op = ps.tile([128, 512], F32, tag="pq", bufs=2, name=f"op_{n}_{row}")
                    for ki, k in enumerate(korder):
                        mm = nc.tensor.matmul(
                            op[:],
                            ats[k][:, 128 * row : 128 * (row + 1)],
                            wo_sb[k][:, 512 * n : 512 * (n + 1)],
                            start=(ki == 0),
                            stop=(ki == 15),
                        )
                        if ki == 0 and prev_phase_last is not None:
                            add_dep_helper(
                                mm.ins,
                                prev_phase_last.ins,
                                sync=False,
                                reason="pin oproj phase order in PE queue",
                            )
                        phase_last = mm
                    ob = sb.tile([128, 512], F32, tag="outsb", bufs=2, name=f"ob_{n}_{row}")
                    nc.vector.tensor_copy(ob[:], op[:])
                    nc.sync.dma_start(
                        out=out[128 * row : 128 * (row + 1), 512 * n : 512 * (n + 1)],
                        in_=ob[:],
                    )
            prev_phase_last = phase_last
            partials = {}
            for n in range(4):
                for row in (2, 3):
                    op = ps.tile([128, 512], F32, tag="pq", bufs=2, name=f"ope_{n}_{row}")
                    for ki, k in enumerate(korder[:8]):
                        mm = nc.tensor.matmul(
                            op[:],
                            ats[k][:, 128 * row : 128 * (row + 1)],
                            wo_sb[k][:, 512 * n : 512 * (n + 1)],
                            start=(ki == 0),
                            stop=(ki == 7),
                        )
                        if ki == 0 and prev_phase_last is not None:
                            add_dep_helper(
                                mm.ins,
                                prev_phase_last.ins,
                                sync=False,
                                reason="pin oproj phase order in PE queue",
                            )
                        prev_phase_last = mm
                    pe_sb = sb.tile([128, 512], BF16, tag="pesb", bufs=8, name=f"pe_{n}_{row}")
                    nc.vector.tensor_copy(pe_sb[:], op[:])
                    partials[(n, row)] = pe_sb
            for n in range(4):
                for row in (2, 3):
                    op = ps.tile([128, 512], F32, tag="pq", bufs=2, name=f"opo_{n}_{row}")
                    for ki, k in enumerate(korder[8:]):
                        mm = nc.tensor.matmul(
                            op[:],
                            ats[k][:, 128 * row : 128 * (row + 1)],
                            wo_sb[k][:, 512 * n : 512 * (n + 1)],
                            start=(ki == 0),
                            stop=(ki == 7),
                        )
                        prev_phase_last = mm
                    ob = sb.tile([128, 512], F32, tag="outsb", bufs=2, name=f"ob_{n}_{row}")
                    nc.vector.tensor_tensor(ob[:], op[:], partials[(n, row)][:], op=ADD)
                    nc.sync.dma_start(
                        out=out[128 * row : 128 * (row + 1), 512 * n : 512 * (n + 1)],
                        in_=ob[:],
                    )

    nc.finalize()
    return nc


_NC_CACHE = None


def _get_nc():
    global _NC_CACHE
    if _NC_CACHE is None:
        _NC_CACHE = build()
    return _NC_CACHE


def _prep_inputs(x, freqs_cis, mask, wq, wk, wv, wo):
    """Host-side sharding / layout prep. Returns per-core input maps."""
    bf16 = ml_dtypes.bfloat16
    xt = np.ascontiguousarray(x.reshape(ROWS, D).T.astype(bf16))  # [D, B*S]
    cos = np.ascontiguousarray(freqs_cis[:, :, 0])  # [S, 32]
    sin = np.ascontiguousarray(freqs_cis[:, :, 1])
    c64 = np.concatenate([cos, cos], axis=1)  # [S, 64]
    s64 = np.concatenate([-sin, sin], axis=1)
    ropec = np.ascontiguousarray(c64.astype(bf16))  # [S, 64]
    ropes = np.ascontiguousarray(s64.astype(bf16))
    # causal 0/1 keep-triangle (keep iff q-col >= kpos-row), replicated twice so
    # one strided multiply masks both chunks of a diagonal group. Derived from
    # the mask input: maskm[r, c] = keep(mask[c, r]) for the leading 128x128.
    tri = (mask[0:128, 0:128].T > -1.0).astype(bf16)
    maskm = np.ascontiguousarray(np.concatenate([tri, tri], axis=1))
    perm = np.concatenate([np.arange(0, 64, 2), np.arange(1, 64, 2)])  # de-interleave
    wo_c = np.ascontiguousarray(wo.astype(bf16))

    in_maps = []
    for c in range(N_CORES):
        heads = range(HPC * c, HPC * (c + 1))
        kv = c // 2
        wq_c = np.concatenate([wq[:, 64 * h + perm] for h in heads], axis=1)
        wk_c = wk[:, 64 * kv + perm]
        wv_c = wv[:, 64 * kv : 64 * (kv + 1)]
        wqkv_c = np.ascontiguousarray(
            np.concatenate([wq_c, wk_c, wv_c], axis=1).astype(bf16)
        )
        in_maps.append(
            {
                "xt": xt,
                "wqkv": wqkv_c,
                "wo": wo_c,
                "ropec": ropec,
                "ropes": ropes,
                "maskm": maskm,
            }
        )
    return in_maps


def kernel(x, freqs_cis, mask, wq, wk, wv, wo, _trace=False, _trace_kwargs=None):
    nc = _get_nc()
    in_maps = _prep_inputs(
        np.asarray(x, np.float32),
        np.asarray(freqs_cis, np.float32),
        np.asarray(mask, np.float32),
        np.asarray(wq, np.float32),
        np.asarray(wk, np.float32),
        np.asarray(wv, np.float32),
        np.asarray(wo, np.float32),
    )
    kwargs = {}
    if _trace:
        kwargs["trace"] = True
        if _trace_kwargs:
            kwargs.update(_trace_kwargs)
    res = run_bass_kernel_spmd(nc, in_maps, core_ids=list(range(N_CORES)), **kwargs)
    full = np.empty((B, S, D), np.float32)
    for c in range(N_CORES):
        oc = res.results[c]["out"]
        full[0, RPC * c : RPC * (c + 1)] = oc[0:RPC]
        full[1, RPC * c : RPC * (c + 1)] = oc[RPC : 2 * RPC]
    if _trace:
        kernel.last_results = res
    return full


if __name__ == "__main__":
    print("building...")
    nc = _get_nc()
    print("built")
